# revision 19
# baseline (speedup 1.0000x reference)
"""Causal bag-of-words pooling (running causal mean) on 8 trn2 NeuronCores.

y[b, t, :] = mean(x[b, :t+1, :])  for x of shape (8, 4096, 1024) fp32.

Sharding: data-parallel over B — core i handles batch element i.

v4 (best measured): bf16 I/O (host converts; rel-err gate 2e-2, this
lands ~4e-3) halves HBM traffic to 16 MB/core; the per-block serial
carry chain of the baseline is replaced by a chain-free two-phase
decomposition; redundant LDWEIGHTS are removed so paired matmuls run
back-to-back:

  Phase A (per 8-block group): block totals via accumulating matmuls
      with one-hot lhsT slices (E8): totA[b, :] = sum of block b's rows.
  Phase B (per group): one matmul vs UT9c turns the 8 totals into 9 carr
      rows (row 0 = next group's carry-in, row b+1 = carry for local
      block b); a second matmul (ONE9) adds the previous group's
      carry-in; one extract per chunk -> carr_sb (bf16).
  Main: MM1 (UT128 within-block cumsum) + MM2 (SEL9 row-select lhsT
      broadcasts carr row b+1) accumulate in PSUM, emitted in block
      PAIRS so the 4 MM1s share one ut LDWEIGHTS and each block's 2
      MM2s share one sel LDWEIGHTS (via the dedup post-pass).
  Evacuation: scaled copy (per-row 1/(t+1) AP) from PSUM to the bf16
      output tile, alternating ScalarE/VectorE by (block+chunk) parity.

Data movement: all DMA via gpsimd SWDGE on 4 parallel queues, full-128-
partition transfers only; input loads all emitted first (group 0 split
4-way across the rings for an early compute start); 1 MB output
transfers.
"""

import sys

import numpy as np

if "/opt/trn_rl_repo" not in sys.path:
    sys.path.insert(0, "/opt/trn_rl_repo")

B, T, C = 8, 4096, 1024
TB = 128                  # rows per block (partition dim)
NB = T // TB              # 32 blocks
FJ = 512                  # matmul moving free dim (PSUM bank = 512 fp32)
NJ = C // FJ              # 2 chunks
GS = 8                    # blocks per carry group
NG = NB // GS             # 4 groups
XIN = 8                   # blocks per input tile
XOUT = 4                  # blocks per output DMA (1 MB bf16 transfers)

_CACHE: dict = {}


def _swq(inst, qnum: int):
    """Route a SWDGE DMA onto qPoolDynamic{qnum} (parallel SWDGE rings)."""
    if qnum:
        inst.ins.queue = f"qPoolDynamic{qnum}"
    return inst


def _dedup_ldweights(nc):
    """Remove InstLdweights whose weights AP + tile_position match the
    previous LDWEIGHTS on the PE stream (only matmuls in between): the
    PE array already holds those weights, and the redundant load both
    costs ~107 ns and breaks back-to-back matmul fill/drain overlap."""
    import concourse.mybir as mybir

    def fp(inst):
        ap = inst.ins[0]
        return (ap.memref, ap.offset, str(ap.ap), str(ap.dtype),
                str(getattr(inst, "tile_position", None)))

    referenced = set()
    for f in nc.m.functions:
        for blk in f.blocks:
            for inst in blk.instructions:
                for nm in inst.sync_dependency_names():
                    referenced.add(nm)
                for nm in inst.nosync_dependency_names():
                    referenced.add(nm)

    removed = 0
    for f in nc.m.functions:
        for blk in f.blocks:
            last_fp = None
            to_remove = []
            for inst in blk.instructions:
                if getattr(inst, "engine", None) != mybir.EngineType.PE:
                    continue
                tn = type(inst).__name__
                if tn == "InstLdweights":
                    cur = fp(inst)
                    if cur == last_fp and inst.name not in referenced:
                        to_remove.append(inst)
                    else:
                        last_fp = cur
                elif tn != "InstMatmult":
                    last_fp = None
            for inst in to_remove:
                blk.instructions.remove(inst)
                removed += 1
    return removed


def _consts():
    import ml_dtypes

    bf16 = ml_dtypes.bfloat16
    # ut128[s, t] = 1 if s <= t : lhsT of the within-block cumsum matmul.
    ut128 = np.triu(np.ones((TB, TB), dtype=np.float32)).astype(bf16)
    # e8[:, 8b:8b+8] is the phase-A lhsT for local block b: col b ones.
    e8 = np.zeros((TB, GS * GS), dtype=np.float32)
    for b in range(GS):
        e8[:, GS * b + b] = 1.0
    e8 = e8.astype(bf16)
    # ut9c[b', 0] = 1 (full group total -> next group's carry-in);
    # ut9c[b', i] = 1 if b' < i-1 (strict prefix for local block i-1).
    ut9c = np.zeros((GS, GS + 1), dtype=np.float32)
    ut9c[:, 0] = 1.0
    for i in range(1, GS + 1):
        ut9c[:i - 1, i] = 1.0
    ut9c = ut9c.astype(bf16)
    one9 = np.ones((1, GS + 1), dtype=np.float32).astype(bf16)
    # sel9[:, 128b:128b+128]: row b+1 ones -> MM2 broadcasts carr row b+1.
    sel9 = np.zeros((GS + 1, GS * TB), dtype=np.float32)
    for b in range(GS):
        sel9[b + 1, TB * b:TB * (b + 1)] = 1.0
    sel9 = sel9.astype(bf16)
    # recip[p, k] = 1 / (k*TB + p + 1)
    t = (np.arange(NB)[None, :] * TB + np.arange(TB)[:, None] + 1).astype(np.float32)
    recip = (np.float32(1.0) / t).astype(np.float32)
    return ut128, e8, ut9c, one9, sel9, recip


def _build():
    from concourse import bacc, tile
    import concourse.mybir as mybir

    f32 = mybir.dt.float32
    bf16 = mybir.dt.bfloat16

    nc = bacc.Bacc(
        "TRN2",
        target_bir_lowering=False,
        debug=False,
        enable_asserts=False,
        num_devices=B,
        num_swdge_queues=4,
    )

    x = nc.dram_tensor("x", [T, C], bf16, kind="ExternalInput").ap()
    ut128 = nc.dram_tensor("ut128", [TB, TB], bf16, kind="ExternalInput").ap()
    e8 = nc.dram_tensor("e8", [TB, GS * GS], bf16, kind="ExternalInput").ap()
    ut9c = nc.dram_tensor("ut9c", [GS, GS + 1], bf16, kind="ExternalInput").ap()
    one9 = nc.dram_tensor("one9", [1, GS + 1], bf16, kind="ExternalInput").ap()
    sel9 = nc.dram_tensor("sel9", [GS + 1, GS * TB], bf16, kind="ExternalInput").ap()
    recip = nc.dram_tensor("recip", [TB, NB], f32, kind="ExternalInput").ap()
    y = nc.dram_tensor("y", [T, C], bf16, kind="ExternalOutput").ap()

    with tile.TileContext(nc) as tc:
        with (
            tc.tile_pool(name="consts", bufs=1) as consts,
            tc.tile_pool(name="xin", bufs=4) as xin,
            tc.tile_pool(name="carr", bufs=2) as carrp,
            tc.tile_pool(name="outp", bufs=4) as outp,
            tc.tile_pool(name="psM", bufs=6, space="PSUM") as psM,
            tc.tile_pool(name="psA", bufs=2, space="PSUM") as psA,
        ):
            ut_t = consts.tile([TB, TB], bf16, tag="ut")
            nc.sync.dma_start(ut_t[:], ut128[:])
            e8_t = consts.tile([TB, GS * GS], bf16, tag="e8")
            nc.sync.dma_start(e8_t[:], e8[:])
            ut9_t = consts.tile([GS, GS + 1], bf16, tag="ut9")
            nc.sync.dma_start(ut9_t[:], ut9c[:])
            one9_t = consts.tile([1, GS + 1], bf16, tag="one9")
            nc.sync.dma_start(one9_t[:], one9[:])
            sel_t = consts.tile([GS + 1, GS * TB], bf16, tag="sel")
            nc.sync.dma_start(sel_t[:], sel9[:])
            rec_t = consts.tile([TB, NB], f32, tag="rec")
            nc.sync.dma_start(rec_t[:], recip[:])

            xts = []
            for g in range(NB // XIN):
                xt = xin.tile([TB, XIN * C], bf16, tag="x", name=f"x{g}")
                nsplit = 4 if g == 0 else (2 if g == 1 else 1)
                h = XIN // nsplit
                for i in range(nsplit):
                    _swq(
                        nc.gpsimd.dma_start(
                            xt[:, i * h * C:(i + 1) * h * C].rearrange(
                                "p (f c) -> p f c", f=h
                            ),
                            x[(g * XIN + i * h) * TB:(g * XIN + (i + 1) * h) * TB, :]
                            .rearrange("(f p) c -> p f c", f=h),
                        ),
                        (g + i) % 4,
                    )
                xts.append(xt)

            def xsl(k, j):
                return xts[k // XIN][
                    :, (k % XIN) * C + j * FJ:(k % XIN) * C + (j + 1) * FJ
                ]

            carrs = [None] * NG
            ots = {}

            def phase_a(g):
                """Block totals of group g -> rows [0:8] of a [9, FJ]
                psum tile per chunk (the same tile is later reused for
                the carr matmuls, keeping the PSUM budget at 8 banks)."""
                tot = [
                    psA.tile([GS + 1, FJ], f32, tag="totA", name="tot")
                    for _ in range(NJ)
                ]
                for b in range(GS):
                    for j in range(NJ):
                        nc.tensor.matmul(
                            tot[j][0:GS, :],
                            e8_t[:, GS * b:GS * (b + 1)],
                            xsl(GS * g + b, j),
                            start=(b == 0),
                            stop=(b == GS - 1),
                        )
                return tot

            def phase_b(g, tot):
                """Totals -> carr rows: row 0 = next group carry-in,
                row b+1 = carry for local block b. Returns bf16 SBUF."""
                tot_sb = carrp.tile([GS, NJ * FJ], bf16, tag="totS", name="tots")
                for j in range(NJ):
                    oc = tot_sb[:, j * FJ:(j + 1) * FJ]
                    if j == 0:
                        nc.scalar.copy(oc, tot[j][0:GS, :])
                    else:
                        nc.vector.tensor_copy(oc, tot[j][0:GS, :])
                carr_sb = carrp.tile([GS + 1, NJ * FJ], bf16, tag="carrS", name="carrs")
                for j in range(NJ):
                    cps = tot[j]  # reuse the totals tile (WAR-serialized)
                    nc.tensor.matmul(
                        cps[:],
                        ut9_t[:],
                        tot_sb[:, j * FJ:(j + 1) * FJ],
                        start=True,
                        stop=(g == 0),
                    )
                    if g > 0:
                        nc.tensor.matmul(
                            cps[:],
                            one9_t[:],
                            carrs[g - 1][0:1, j * FJ:(j + 1) * FJ],
                            start=False,
                            stop=True,
                        )
                    oc = carr_sb[:, j * FJ:(j + 1) * FJ]
                    if j == 0:
                        nc.vector.tensor_copy(oc, cps[:])
                    else:
                        nc.scalar.copy(oc, cps[:])
                carrs[g] = carr_sb

            def main(g):
                """MM1 + MM2 + evacuation + store for group g, emitted in
                block PAIRS so all 4 MM1s share one ut LDWEIGHTS and each
                block's 2 MM2s share one sel9 LDWEIGHTS (after dedup)."""
                for b0 in range(0, GS, 2):
                    pss = {}
                    for b in (b0, b0 + 1):
                        k = GS * g + b
                        for j in range(NJ):
                            ps = psM.tile([TB, FJ], f32, tag="psM", name="ps")
                            pss[(b, j)] = ps
                            nc.tensor.matmul(
                                ps[:],
                                ut_t[:],
                                xsl(k, j),
                                start=True,
                                stop=(k == 0),
                            )
                    for b in (b0, b0 + 1):
                        k = GS * g + b
                        if k == 0:
                            continue
                        for j in range(NJ):
                            nc.tensor.matmul(
                                pss[(b, j)][:],
                                sel_t[:, TB * b:TB * (b + 1)],
                                carrs[g][:, j * FJ:(j + 1) * FJ],
                                start=False,
                                stop=True,
                            )
                    for b in (b0, b0 + 1):
                        k = GS * g + b
                        og = k // XOUT
                        if k % XOUT == 0:
                            ots[og] = outp.tile(
                                [TB, XOUT * C], bf16, tag="out", name="ot"
                            )
                        ot = ots[og]
                        boff = (k % XOUT) * C
                        for j in range(NJ):
                            oc = ot[:, boff + j * FJ:boff + (j + 1) * FJ]
                            ps = pss[(b, j)]
                            if (k + j) % 2 == 0:
                                nc.scalar.mul(oc, ps[:], rec_t[:, k:k + 1])
                            else:
                                nc.vector.tensor_scalar_mul(
                                    oc, ps[:], rec_t[:, k:k + 1]
                                )
                        if k % XOUT == XOUT - 1:
                            _swq(
                                nc.gpsimd.dma_start(
                                    y[og * XOUT * TB:(og + 1) * XOUT * TB, :]
                                    .rearrange("(f p) c -> p f c", f=XOUT),
                                    ot[:].rearrange("p (f c) -> p f c", f=XOUT),
                                ),
                                (og + 1) % 4,
                            )

            # Interleave: A(g+1) between B(g) and M(g), so phase-B
            # extract latency hides under main-pass matmuls.
            tot = phase_a(0)
            phase_b(0, tot)
            for g in range(NG):
                if g + 1 < NG:
                    tot = phase_a(g + 1)
                main(g)
                if g + 1 < NG:
                    phase_b(g + 1, tot)

    n_removed = _dedup_ldweights(nc)
    sys.stderr.write(f"[kernel] deduped {n_removed} LDWEIGHTS\n")
    nc.compile()

    from concourse.bass_interp import get_hw_module

    nc.m = get_hw_module(nc.m)
    return nc


def _run(x_full: np.ndarray, trace: bool = False):
    import ml_dtypes
    from concourse.bass_utils import run_bass_kernel_spmd

    if "nc" not in _CACHE:
        _CACHE["nc"] = _build()
    nc = _CACHE["nc"]

    ut128, e8, ut9c, one9, sel9, recip = _consts()
    x_full = np.asarray(x_full)
    in_maps = [
        {
            "x": np.ascontiguousarray(x_full[i]).astype(ml_dtypes.bfloat16),
            "ut128": ut128,
            "e8": e8,
            "ut9c": ut9c,
            "one9": one9,
            "sel9": sel9,
            "recip": recip,
        }
        for i in range(B)
    ]
    res = run_bass_kernel_spmd(nc, in_maps, core_ids=list(range(B)), trace=trace)
    out = np.stack(
        [np.asarray(res.results[i]["y"]).astype(np.float32) for i in range(B)],
        axis=0,
    )
    return out, res


def kernel(x: np.ndarray) -> np.ndarray:
    out, _ = _run(x, trace=False)
    return out


# revision 27
# speedup vs baseline: 1.0579x; 1.0579x over previous
"""Causal bag-of-words pooling (running causal mean) on 8 trn2 NeuronCores.

y[b, t, :] = mean(x[b, :t+1, :])  for x of shape (8, 4096, 1024) fp32.

Sharding: data-parallel over B — core i handles batch element i.

v4 (best measured): bf16 I/O (host converts; rel-err gate 2e-2, this
lands ~4e-3) halves HBM traffic to 16 MB/core; the per-block serial
carry chain of the baseline is replaced by a chain-free two-phase
decomposition; redundant LDWEIGHTS are removed so paired matmuls run
back-to-back:

  Phase A (per 8-block group): block totals via accumulating matmuls
      with one-hot lhsT slices (E8): totA[b, :] = sum of block b's rows.
  Phase B (per group): one matmul vs UT9c turns the 8 totals into 9 carr
      rows (row 0 = next group's carry-in, row b+1 = carry for local
      block b); a second matmul (ONE9) adds the previous group's
      carry-in; one extract per chunk -> carr_sb (bf16).
  Main: MM1 (UT128 within-block cumsum) + MM2 (SEL9 row-select lhsT
      broadcasts carr row b+1) accumulate in PSUM, emitted in block
      PAIRS so the 4 MM1s share one ut LDWEIGHTS and each block's 2
      MM2s share one sel LDWEIGHTS (via the dedup post-pass).
  Evacuation: scaled copy (per-row 1/(t+1) AP) from PSUM to the bf16
      output tile, alternating ScalarE/VectorE by (block+chunk) parity.

Data movement: all DMA via gpsimd SWDGE on 4 parallel queues, full-128-
partition transfers only; input loads all emitted first (group 0 split
4-way across the rings for an early compute start); 1 MB output
transfers.
"""

import sys

import numpy as np

if "/opt/trn_rl_repo" not in sys.path:
    sys.path.insert(0, "/opt/trn_rl_repo")

B, T, C = 8, 4096, 1024
TB = 128                  # rows per block (partition dim)
NB = T // TB              # 32 blocks
FJ = 512                  # matmul moving free dim (PSUM bank = 512 fp32)
NJ = C // FJ              # 2 chunks
GS = 8                    # blocks per carry group
NG = NB // GS             # 4 groups
XIN = 8                   # blocks per input tile
XOUT = 4                  # blocks per output DMA (1 MB bf16 transfers)

_CACHE: dict = {}


def _swq(inst, qnum: int):
    """Route a SWDGE DMA onto qPoolDynamic{qnum} (parallel SWDGE rings)."""
    if qnum:
        inst.ins.queue = f"qPoolDynamic{qnum}"
    return inst


def _dedup_ldweights(nc):
    """Remove InstLdweights whose weights AP + tile_position match the
    previous LDWEIGHTS on the PE stream (only matmuls in between): the
    PE array already holds those weights, and the redundant load both
    costs ~107 ns and breaks back-to-back matmul fill/drain overlap."""
    import concourse.mybir as mybir

    def fp(inst):
        ap = inst.ins[0]
        return (ap.memref, ap.offset, str(ap.ap), str(ap.dtype),
                str(getattr(inst, "tile_position", None)))

    referenced = set()
    for f in nc.m.functions:
        for blk in f.blocks:
            for inst in blk.instructions:
                for nm in inst.sync_dependency_names():
                    referenced.add(nm)
                for nm in inst.nosync_dependency_names():
                    referenced.add(nm)

    removed = 0
    for f in nc.m.functions:
        for blk in f.blocks:
            last_fp = None
            to_remove = []
            for inst in blk.instructions:
                if getattr(inst, "engine", None) != mybir.EngineType.PE:
                    continue
                tn = type(inst).__name__
                if tn == "InstLdweights":
                    cur = fp(inst)
                    if cur == last_fp and inst.name not in referenced:
                        to_remove.append(inst)
                    else:
                        last_fp = cur
                elif tn != "InstMatmult":
                    last_fp = None
            for inst in to_remove:
                blk.instructions.remove(inst)
                removed += 1
    return removed


def _consts():
    import ml_dtypes

    bf16 = ml_dtypes.bfloat16
    # ut128[s, t] = 1 if s <= t : lhsT of the within-block cumsum matmul.
    ut128 = np.triu(np.ones((TB, TB), dtype=np.float32)).astype(bf16)
    # e8[:, 8b:8b+8] is the phase-A lhsT for local block b: col b ones.
    e8 = np.zeros((TB, GS * GS), dtype=np.float32)
    for b in range(GS):
        e8[:, GS * b + b] = 1.0
    e8 = e8.astype(bf16)
    # ut9c[b', 0] = 1 (full group total -> next group's carry-in);
    # ut9c[b', i] = 1 if b' < i-1 (strict prefix for local block i-1).
    ut9c = np.zeros((GS, GS + 1), dtype=np.float32)
    ut9c[:, 0] = 1.0
    for i in range(1, GS + 1):
        ut9c[:i - 1, i] = 1.0
    ut9c = ut9c.astype(bf16)
    one9 = np.ones((1, GS + 1), dtype=np.float32).astype(bf16)
    # sel9[:, 128b:128b+128]: row b+1 ones -> MM2 broadcasts carr row b+1.
    sel9 = np.zeros((GS + 1, GS * TB), dtype=np.float32)
    for b in range(GS):
        sel9[b + 1, TB * b:TB * (b + 1)] = 1.0
    sel9 = sel9.astype(bf16)
    # recip[p, k] = 1 / (k*TB + p + 1)
    t = (np.arange(NB)[None, :] * TB + np.arange(TB)[:, None] + 1).astype(np.float32)
    recip = (np.float32(1.0) / t).astype(np.float32)
    return ut128, e8, ut9c, one9, sel9, recip


def _build():
    from concourse import bacc, tile
    import concourse.mybir as mybir

    f32 = mybir.dt.float32
    bf16 = mybir.dt.bfloat16

    nc = bacc.Bacc(
        "TRN2",
        target_bir_lowering=False,
        debug=False,
        enable_asserts=False,
        num_devices=B,
        num_swdge_queues=4,
    )

    x = nc.dram_tensor("x", [T, C], bf16, kind="ExternalInput").ap()
    ut128 = nc.dram_tensor("ut128", [TB, TB], bf16, kind="ExternalInput").ap()
    e8 = nc.dram_tensor("e8", [TB, GS * GS], bf16, kind="ExternalInput").ap()
    ut9c = nc.dram_tensor("ut9c", [GS, GS + 1], bf16, kind="ExternalInput").ap()
    one9 = nc.dram_tensor("one9", [1, GS + 1], bf16, kind="ExternalInput").ap()
    sel9 = nc.dram_tensor("sel9", [GS + 1, GS * TB], bf16, kind="ExternalInput").ap()
    recip = nc.dram_tensor("recip", [TB, NB], f32, kind="ExternalInput").ap()
    y = nc.dram_tensor("y", [T, C], bf16, kind="ExternalOutput").ap()

    with tile.TileContext(nc) as tc:
        with (
            tc.tile_pool(name="consts", bufs=1) as consts,
            tc.tile_pool(name="xin", bufs=4) as xin,
            tc.tile_pool(name="carr", bufs=2) as carrp,
            tc.tile_pool(name="outp", bufs=4) as outp,
            tc.tile_pool(name="psM", bufs=6, space="PSUM") as psM,
            tc.tile_pool(name="psA", bufs=2, space="PSUM") as psA,
        ):
            ut_t = consts.tile([TB, TB], bf16, tag="ut")
            nc.sync.dma_start(ut_t[:], ut128[:])
            e8_t = consts.tile([TB, GS * GS], bf16, tag="e8")
            nc.sync.dma_start(e8_t[:], e8[:])
            ut9_t = consts.tile([GS, GS + 1], bf16, tag="ut9")
            nc.sync.dma_start(ut9_t[:], ut9c[:])
            one9_t = consts.tile([1, GS + 1], bf16, tag="one9")
            nc.sync.dma_start(one9_t[:], one9[:])
            sel_t = consts.tile([GS + 1, GS * TB], bf16, tag="sel")
            nc.sync.dma_start(sel_t[:], sel9[:])
            rec_t = consts.tile([TB, NB], f32, tag="rec")
            nc.sync.dma_start(rec_t[:], recip[:])

            xts = []
            for g in range(NB // XIN):
                xt = xin.tile([TB, XIN * C], bf16, tag="x", name=f"x{g}")
                nsplit = 4 if g == 0 else (2 if g == 1 else 1)
                h = XIN // nsplit
                for i in range(nsplit):
                    _swq(
                        nc.gpsimd.dma_start(
                            xt[:, i * h * C:(i + 1) * h * C].rearrange(
                                "p (f c) -> p f c", f=h
                            ),
                            x[(g * XIN + i * h) * TB:(g * XIN + (i + 1) * h) * TB, :]
                            .rearrange("(f p) c -> p f c", f=h),
                        ),
                        (g + i) % 4,
                    )
                xts.append(xt)

            def xsl(k, j):
                return xts[k // XIN][
                    :, (k % XIN) * C + j * FJ:(k % XIN) * C + (j + 1) * FJ
                ]

            carrs = [None] * NG
            ots = {}

            # PE warmup: dummy matmuls into a scratch PSUM tile while the
            # SWDGE rings spin up (~10 us with an idle PE); without them
            # HAM clocks the first ~3.4 us of real matmuls at 1.2 GHz.
            # FD=256 each: short enough to not delay phase A materially.
            wps = psA.tile([GS + 1, FJ], f32, tag="totA", name="warm")
            for _ in range(40):
                nc.tensor.matmul(
                    wps[0:GS, 0:TB],
                    e8_t[:, 0:GS],
                    ut_t[:],
                    start=True,
                    stop=True,
                    skip_group_check=True,
                )

            def phase_a(g):
                """Block totals of group g -> rows [0:8] of a [9, FJ]
                psum tile per chunk (the same tile is later reused for
                the carr matmuls, keeping the PSUM budget at 8 banks)."""
                tot = [
                    psA.tile([GS + 1, FJ], f32, tag="totA", name="tot")
                    for _ in range(NJ)
                ]
                for b in range(GS):
                    for j in range(NJ):
                        nc.tensor.matmul(
                            tot[j][0:GS, :],
                            e8_t[:, GS * b:GS * (b + 1)],
                            xsl(GS * g + b, j),
                            start=(b == 0),
                            stop=(b == GS - 1),
                        )
                return tot

            def phase_b(g, tot):
                """Totals -> carr rows: row 0 = next group carry-in,
                row b+1 = carry for local block b. Returns bf16 SBUF."""
                tot_sb = carrp.tile([GS, NJ * FJ], bf16, tag="totS", name="tots")
                for j in range(NJ):
                    oc = tot_sb[:, j * FJ:(j + 1) * FJ]
                    if j == 0:
                        nc.scalar.copy(oc, tot[j][0:GS, :])
                    else:
                        nc.vector.tensor_copy(oc, tot[j][0:GS, :])
                carr_sb = carrp.tile([GS + 1, NJ * FJ], bf16, tag="carrS", name="carrs")
                for j in range(NJ):
                    cps = tot[j]  # reuse the totals tile (WAR-serialized)
                    nc.tensor.matmul(
                        cps[:],
                        ut9_t[:],
                        tot_sb[:, j * FJ:(j + 1) * FJ],
                        start=True,
                        stop=(g == 0),
                    )
                    if g > 0:
                        nc.tensor.matmul(
                            cps[:],
                            one9_t[:],
                            carrs[g - 1][0:1, j * FJ:(j + 1) * FJ],
                            start=False,
                            stop=True,
                        )
                    oc = carr_sb[:, j * FJ:(j + 1) * FJ]
                    if j == 0:
                        nc.vector.tensor_copy(oc, cps[:])
                    else:
                        nc.scalar.copy(oc, cps[:])
                carrs[g] = carr_sb

            def main(g, mid=None):
                """MM1 + MM2 + evacuation + store for group g, emitted in
                block PAIRS so all 4 MM1s share one ut LDWEIGHTS and each
                block's 2 MM2s share one sel9 LDWEIGHTS (after dedup).
                `mid()` (phase B of the next group) is emitted after the
                first pair, so its ACT/DVE extracts enqueue ahead of most
                of this group's evacuation backlog."""
                for b0 in range(0, GS, 2):
                    if b0 == 2 and mid is not None:
                        mid()
                    pss = {}
                    for b in (b0, b0 + 1):
                        k = GS * g + b
                        for j in range(NJ):
                            ps = psM.tile([TB, FJ], f32, tag="psM", name="ps")
                            pss[(b, j)] = ps
                            nc.tensor.matmul(
                                ps[:],
                                ut_t[:],
                                xsl(k, j),
                                start=True,
                                stop=(k == 0),
                            )
                    for b in (b0, b0 + 1):
                        k = GS * g + b
                        if k == 0:
                            continue
                        for j in range(NJ):
                            nc.tensor.matmul(
                                pss[(b, j)][:],
                                sel_t[:, TB * b:TB * (b + 1)],
                                carrs[g][:, j * FJ:(j + 1) * FJ],
                                start=False,
                                stop=True,
                            )
                    for b in (b0, b0 + 1):
                        k = GS * g + b
                        og = k // XOUT
                        if k % XOUT == 0:
                            ots[og] = outp.tile(
                                [TB, XOUT * C], bf16, tag="out", name="ot"
                            )
                        ot = ots[og]
                        boff = (k % XOUT) * C
                        for j in range(NJ):
                            oc = ot[:, boff + j * FJ:boff + (j + 1) * FJ]
                            ps = pss[(b, j)]
                            if (k + j) % 2 == 0:
                                nc.scalar.mul(oc, ps[:], rec_t[:, k:k + 1])
                            else:
                                nc.vector.tensor_scalar_mul(
                                    oc, ps[:], rec_t[:, k:k + 1]
                                )
                        if k % XOUT == XOUT - 1:
                            # Last group: 2 half-stores on separate rings
                            # (the final store otherwise runs alone at
                            # ~170 GB/s and adds ~6 us of pure tail).
                            nsp = 2 if og == NB // XOUT - 1 else 1
                            hh = XOUT // nsp
                            for i in range(nsp):
                                _swq(
                                    nc.gpsimd.dma_start(
                                        y[(og * XOUT + i * hh) * TB:
                                          (og * XOUT + (i + 1) * hh) * TB, :]
                                        .rearrange("(f p) c -> p f c", f=hh),
                                        ot[:, i * hh * C:(i + 1) * hh * C]
                                        .rearrange("p (f c) -> p f c", f=hh),
                                    ),
                                    (og + 1 + i) % 4,
                                )

            # A(g+1) before main(g); B(g+1) emitted mid-main(g) so its
            # extracts don't queue behind the whole evacuation backlog
            # (measured 5-7 us of PE stall per group boundary otherwise).
            tot = phase_a(0)
            phase_b(0, tot)
            for g in range(NG):
                if g + 1 < NG:
                    tot = phase_a(g + 1)
                    tcur = tot
                    main(g, mid=lambda gg=g + 1, tt=tcur: phase_b(gg, tt))
                else:
                    main(g)

    n_removed = _dedup_ldweights(nc)
    sys.stderr.write(f"[kernel] deduped {n_removed} LDWEIGHTS\n")
    nc.compile()

    from concourse.bass_interp import get_hw_module

    nc.m = get_hw_module(nc.m)
    return nc


def _run(x_full: np.ndarray, trace: bool = False):
    import ml_dtypes
    from concourse.bass_utils import run_bass_kernel_spmd

    if "nc" not in _CACHE:
        _CACHE["nc"] = _build()
    nc = _CACHE["nc"]

    ut128, e8, ut9c, one9, sel9, recip = _consts()
    x_full = np.asarray(x_full)
    in_maps = [
        {
            "x": np.ascontiguousarray(x_full[i]).astype(ml_dtypes.bfloat16),
            "ut128": ut128,
            "e8": e8,
            "ut9c": ut9c,
            "one9": one9,
            "sel9": sel9,
            "recip": recip,
        }
        for i in range(B)
    ]
    res = run_bass_kernel_spmd(nc, in_maps, core_ids=list(range(B)), trace=trace)
    out = np.stack(
        [np.asarray(res.results[i]["y"]).astype(np.float32) for i in range(B)],
        axis=0,
    )
    return out, res


def kernel(x: np.ndarray) -> np.ndarray:
    out, _ = _run(x, trace=False)
    return out


# revision 29
# speedup vs baseline: 1.1732x; 1.1090x over previous
"""Causal bag-of-words pooling (running causal mean) on 8 trn2 NeuronCores.

y[b, t, :] = mean(x[b, :t+1, :])  for x of shape (8, 4096, 1024) fp32.

Sharding: data-parallel over B — core i handles batch element i.

v4 (best measured): bf16 I/O (host converts; rel-err gate 2e-2, this
lands ~4e-3) halves HBM traffic to 16 MB/core; the per-block serial
carry chain of the baseline is replaced by a chain-free two-phase
decomposition; redundant LDWEIGHTS are removed so paired matmuls run
back-to-back:

  Phase A (per 8-block group): block totals via accumulating matmuls
      with one-hot lhsT slices (E8): totA[b, :] = sum of block b's rows.
  Phase B (per group): one matmul vs UT9c turns the 8 totals into 9 carr
      rows (row 0 = next group's carry-in, row b+1 = carry for local
      block b); a second matmul (ONE9) adds the previous group's
      carry-in; one extract per chunk -> carr_sb (bf16).
  Main: MM1 (UT128 within-block cumsum) + MM2 (SEL9 row-select lhsT
      broadcasts carr row b+1) accumulate in PSUM, emitted in block
      PAIRS so the 4 MM1s share one ut LDWEIGHTS and each block's 2
      MM2s share one sel LDWEIGHTS (via the dedup post-pass).
  Evacuation: scaled copy (per-row 1/(t+1) AP) from PSUM to the bf16
      output tile, alternating ScalarE/VectorE by (block+chunk) parity.

Data movement: all DMA via gpsimd SWDGE on 4 parallel queues, full-128-
partition transfers only; input loads all emitted first (group 0 split
4-way across the rings for an early compute start); 1 MB output
transfers.
"""

import sys

import numpy as np

if "/opt/trn_rl_repo" not in sys.path:
    sys.path.insert(0, "/opt/trn_rl_repo")

B, T, C = 8, 4096, 1024
TB = 128                  # rows per block (partition dim)
NB = T // TB              # 32 blocks
FJ = 512                  # matmul moving free dim (PSUM bank = 512 fp32)
NJ = C // FJ              # 2 chunks
GS = 8                    # blocks per carry group
NG = NB // GS             # 4 groups
XIN = 8                   # blocks per input tile
XOUT = 4                  # blocks per output DMA (1 MB bf16 transfers)

_CACHE: dict = {}


def _swq(inst, qnum: int):
    """Route a SWDGE DMA onto qPoolDynamic{qnum} (parallel SWDGE rings)."""
    if qnum:
        inst.ins.queue = f"qPoolDynamic{qnum}"
    return inst


def _dedup_ldweights(nc):
    """Remove InstLdweights whose weights AP + tile_position match the
    previous LDWEIGHTS on the PE stream (only matmuls in between): the
    PE array already holds those weights, and the redundant load both
    costs ~107 ns and breaks back-to-back matmul fill/drain overlap."""
    import concourse.mybir as mybir

    def fp(inst):
        ap = inst.ins[0]
        return (ap.memref, ap.offset, str(ap.ap), str(ap.dtype),
                str(getattr(inst, "tile_position", None)))

    referenced = set()
    for f in nc.m.functions:
        for blk in f.blocks:
            for inst in blk.instructions:
                for nm in inst.sync_dependency_names():
                    referenced.add(nm)
                for nm in inst.nosync_dependency_names():
                    referenced.add(nm)

    removed = 0
    for f in nc.m.functions:
        for blk in f.blocks:
            last_fp = None
            to_remove = []
            for inst in blk.instructions:
                if getattr(inst, "engine", None) != mybir.EngineType.PE:
                    continue
                tn = type(inst).__name__
                if tn == "InstLdweights":
                    cur = fp(inst)
                    if cur == last_fp and inst.name not in referenced:
                        to_remove.append(inst)
                    else:
                        last_fp = cur
                elif tn != "InstMatmult":
                    last_fp = None
            for inst in to_remove:
                blk.instructions.remove(inst)
                removed += 1
    return removed


def _consts():
    import ml_dtypes

    bf16 = ml_dtypes.bfloat16
    # ut128[s, t] = 1 if s <= t : lhsT of the within-block cumsum matmul.
    ut128 = np.triu(np.ones((TB, TB), dtype=np.float32)).astype(bf16)
    # e8[:, 8b:8b+8] is the phase-A lhsT for local block b: col b ones.
    e8 = np.zeros((TB, GS * GS), dtype=np.float32)
    for b in range(GS):
        e8[:, GS * b + b] = 1.0
    e8 = e8.astype(bf16)
    # ut9c[b', 0] = 1 (full group total -> next group's carry-in);
    # ut9c[b', i] = 1 if b' < i-1 (strict prefix for local block i-1).
    ut9c = np.zeros((GS, GS + 1), dtype=np.float32)
    ut9c[:, 0] = 1.0
    for i in range(1, GS + 1):
        ut9c[:i - 1, i] = 1.0
    ut9c = ut9c.astype(bf16)
    one9 = np.ones((1, GS + 1), dtype=np.float32).astype(bf16)
    # sel9[:, 128b:128b+128]: row b+1 ones -> MM2 broadcasts carr row b+1.
    sel9 = np.zeros((GS + 1, GS * TB), dtype=np.float32)
    for b in range(GS):
        sel9[b + 1, TB * b:TB * (b + 1)] = 1.0
    sel9 = sel9.astype(bf16)
    # recip[p, k] = 1 / (k*TB + p + 1)
    t = (np.arange(NB)[None, :] * TB + np.arange(TB)[:, None] + 1).astype(np.float32)
    recip = (np.float32(1.0) / t).astype(np.float32)
    return ut128, e8, ut9c, one9, sel9, recip


def _build():
    from concourse import bacc, tile
    import concourse.mybir as mybir

    f32 = mybir.dt.float32
    bf16 = mybir.dt.bfloat16

    nc = bacc.Bacc(
        "TRN2",
        target_bir_lowering=False,
        debug=False,
        enable_asserts=False,
        num_devices=B,
        num_swdge_queues=4,
    )

    x = nc.dram_tensor("x", [T, C], bf16, kind="ExternalInput").ap()
    ut128 = nc.dram_tensor("ut128", [TB, TB], bf16, kind="ExternalInput").ap()
    e8 = nc.dram_tensor("e8", [TB, GS * GS], bf16, kind="ExternalInput").ap()
    ut9c = nc.dram_tensor("ut9c", [GS, GS + 1], bf16, kind="ExternalInput").ap()
    one9 = nc.dram_tensor("one9", [1, GS + 1], bf16, kind="ExternalInput").ap()
    sel9 = nc.dram_tensor("sel9", [GS + 1, GS * TB], bf16, kind="ExternalInput").ap()
    recip = nc.dram_tensor("recip", [TB, NB], f32, kind="ExternalInput").ap()
    y = nc.dram_tensor("y", [T, C], bf16, kind="ExternalOutput").ap()

    with tile.TileContext(nc) as tc:
        with (
            tc.tile_pool(name="consts", bufs=1) as consts,
            tc.tile_pool(name="xin", bufs=4) as xin,
            tc.tile_pool(name="carr", bufs=2) as carrp,
            tc.tile_pool(name="outp", bufs=4) as outp,
            tc.tile_pool(name="psM", bufs=6, space="PSUM") as psM,
            tc.tile_pool(name="psA", bufs=2, space="PSUM") as psA,
        ):
            ut_t = consts.tile([TB, TB], bf16, tag="ut")
            nc.sync.dma_start(ut_t[:], ut128[:])
            e8_t = consts.tile([TB, GS * GS], bf16, tag="e8")
            nc.sync.dma_start(e8_t[:], e8[:])
            ut9_t = consts.tile([GS, GS + 1], bf16, tag="ut9")
            nc.sync.dma_start(ut9_t[:], ut9c[:])
            one9_t = consts.tile([1, GS + 1], bf16, tag="one9")
            nc.sync.dma_start(one9_t[:], one9[:])
            sel_t = consts.tile([GS + 1, GS * TB], bf16, tag="sel")
            nc.sync.dma_start(sel_t[:], sel9[:])
            rec_t = consts.tile([TB, NB], f32, tag="rec")
            nc.sync.dma_start(rec_t[:], recip[:])

            xts = []
            for g in range(NB // XIN):
                xt = xin.tile([TB, XIN * C], bf16, tag="x", name=f"x{g}")
                nsplit = 4 if g == 0 else 2
                h = XIN // nsplit
                for i in range(nsplit):
                    _swq(
                        nc.gpsimd.dma_start(
                            xt[:, i * h * C:(i + 1) * h * C].rearrange(
                                "p (f c) -> p f c", f=h
                            ),
                            x[(g * XIN + i * h) * TB:(g * XIN + (i + 1) * h) * TB, :]
                            .rearrange("(f p) c -> p f c", f=h),
                        ),
                        (2 * g + i) % 4,
                    )
                xts.append(xt)

            def xsl(k, j):
                return xts[k // XIN][
                    :, (k % XIN) * C + j * FJ:(k % XIN) * C + (j + 1) * FJ
                ]

            carrs = [None] * NG
            ots = {}

            # PE warmup: dummy matmuls into a scratch PSUM tile while the
            # SWDGE rings spin up (~10 us with an idle PE); without them
            # HAM clocks the first ~3.4 us of real matmuls at 1.2 GHz.
            # FD=256 each: short enough to not delay phase A materially.
            wps = psA.tile([GS + 1, FJ], f32, tag="totA", name="warm")
            for _ in range(40):
                nc.tensor.matmul(
                    wps[0:GS, 0:TB],
                    e8_t[:, 0:GS],
                    ut_t[:],
                    start=True,
                    stop=True,
                    skip_group_check=True,
                )

            def phase_a(g):
                """Block totals of group g -> rows [0:8] of a [9, FJ]
                psum tile per chunk (the same tile is later reused for
                the carr matmuls, keeping the PSUM budget at 8 banks)."""
                tot = [
                    psA.tile([GS + 1, FJ], f32, tag="totA", name="tot")
                    for _ in range(NJ)
                ]
                for b in range(GS):
                    for j in range(NJ):
                        nc.tensor.matmul(
                            tot[j][0:GS, :],
                            e8_t[:, GS * b:GS * (b + 1)],
                            xsl(GS * g + b, j),
                            start=(b == 0),
                            stop=(b == GS - 1),
                        )
                return tot

            def phase_b(g, tot):
                """Totals -> carr rows: row 0 = next group carry-in,
                row b+1 = carry for local block b. Returns bf16 SBUF."""
                tot_sb = carrp.tile([GS, NJ * FJ], bf16, tag="totS", name="tots")
                for j in range(NJ):
                    oc = tot_sb[:, j * FJ:(j + 1) * FJ]
                    if j == 0:
                        nc.scalar.copy(oc, tot[j][0:GS, :])
                    else:
                        nc.vector.tensor_copy(oc, tot[j][0:GS, :])
                carr_sb = carrp.tile([GS + 1, NJ * FJ], bf16, tag="carrS", name="carrs")
                for j in range(NJ):
                    cps = tot[j]  # reuse the totals tile (WAR-serialized)
                    nc.tensor.matmul(
                        cps[:],
                        ut9_t[:],
                        tot_sb[:, j * FJ:(j + 1) * FJ],
                        start=True,
                        stop=(g == 0),
                    )
                    if g > 0:
                        nc.tensor.matmul(
                            cps[:],
                            one9_t[:],
                            carrs[g - 1][0:1, j * FJ:(j + 1) * FJ],
                            start=False,
                            stop=True,
                        )
                    oc = carr_sb[:, j * FJ:(j + 1) * FJ]
                    if j == 0:
                        nc.vector.tensor_copy(oc, cps[:])
                    else:
                        nc.scalar.copy(oc, cps[:])
                carrs[g] = carr_sb

            def main(g, mid=None):
                """MM1 + MM2 + evacuation + store for group g, emitted in
                block PAIRS so all 4 MM1s share one ut LDWEIGHTS and each
                block's 2 MM2s share one sel9 LDWEIGHTS (after dedup).
                `mid()` (phase B of the next group) is emitted after the
                first pair, so its ACT/DVE extracts enqueue ahead of most
                of this group's evacuation backlog."""
                for b0 in range(0, GS, 2):
                    if b0 == 2 and mid is not None:
                        mid()
                    pss = {}
                    for b in (b0, b0 + 1):
                        k = GS * g + b
                        for j in range(NJ):
                            ps = psM.tile([TB, FJ], f32, tag="psM", name="ps")
                            pss[(b, j)] = ps
                            nc.tensor.matmul(
                                ps[:],
                                ut_t[:],
                                xsl(k, j),
                                start=True,
                                stop=(k == 0),
                            )
                    for b in (b0, b0 + 1):
                        k = GS * g + b
                        if k == 0:
                            continue
                        for j in range(NJ):
                            nc.tensor.matmul(
                                pss[(b, j)][:],
                                sel_t[:, TB * b:TB * (b + 1)],
                                carrs[g][:, j * FJ:(j + 1) * FJ],
                                start=False,
                                stop=True,
                            )
                    for b in (b0, b0 + 1):
                        k = GS * g + b
                        og = k // XOUT
                        if k % XOUT == 0:
                            ots[og] = outp.tile(
                                [TB, XOUT * C], bf16, tag="out", name="ot"
                            )
                        ot = ots[og]
                        boff = (k % XOUT) * C
                        for j in range(NJ):
                            oc = ot[:, boff + j * FJ:boff + (j + 1) * FJ]
                            ps = pss[(b, j)]
                            if (k + j) % 2 == 0:
                                nc.scalar.mul(oc, ps[:], rec_t[:, k:k + 1])
                            else:
                                nc.vector.tensor_scalar_mul(
                                    oc, ps[:], rec_t[:, k:k + 1]
                                )
                        if k % XOUT == XOUT - 1:
                            # Last group: 2 half-stores on separate rings
                            # (the final store otherwise runs alone at
                            # ~170 GB/s and adds ~6 us of pure tail).
                            nsp = 2 if og == NB // XOUT - 1 else 1
                            hh = XOUT // nsp
                            for i in range(nsp):
                                _swq(
                                    nc.gpsimd.dma_start(
                                        y[(og * XOUT + i * hh) * TB:
                                          (og * XOUT + (i + 1) * hh) * TB, :]
                                        .rearrange("(f p) c -> p f c", f=hh),
                                        ot[:, i * hh * C:(i + 1) * hh * C]
                                        .rearrange("p (f c) -> p f c", f=hh),
                                    ),
                                    (og + 1 + i) % 4,
                                )

            # A(g+1) before main(g); B(g+1) emitted mid-main(g) so its
            # extracts don't queue behind the whole evacuation backlog
            # (measured 5-7 us of PE stall per group boundary otherwise).
            tot = phase_a(0)
            phase_b(0, tot)
            for g in range(NG):
                if g + 1 < NG:
                    tot = phase_a(g + 1)
                    tcur = tot
                    main(g, mid=lambda gg=g + 1, tt=tcur: phase_b(gg, tt))
                else:
                    main(g)

    n_removed = _dedup_ldweights(nc)
    sys.stderr.write(f"[kernel] deduped {n_removed} LDWEIGHTS\n")
    nc.compile()

    from concourse.bass_interp import get_hw_module

    nc.m = get_hw_module(nc.m)
    return nc


def _run(x_full: np.ndarray, trace: bool = False):
    import ml_dtypes
    from concourse.bass_utils import run_bass_kernel_spmd

    if "nc" not in _CACHE:
        _CACHE["nc"] = _build()
    nc = _CACHE["nc"]

    ut128, e8, ut9c, one9, sel9, recip = _consts()
    x_full = np.asarray(x_full)
    in_maps = [
        {
            "x": np.ascontiguousarray(x_full[i]).astype(ml_dtypes.bfloat16),
            "ut128": ut128,
            "e8": e8,
            "ut9c": ut9c,
            "one9": one9,
            "sel9": sel9,
            "recip": recip,
        }
        for i in range(B)
    ]
    res = run_bass_kernel_spmd(nc, in_maps, core_ids=list(range(B)), trace=trace)
    out = np.stack(
        [np.asarray(res.results[i]["y"]).astype(np.float32) for i in range(B)],
        axis=0,
    )
    return out, res


def kernel(x: np.ndarray) -> np.ndarray:
    out, _ = _run(x, trace=False)
    return out
